# revision 15
# baseline (speedup 1.0000x reference)
"""Trainium2 Bass kernel for nn_CrossAttention (b=2, s1=2048, s2=3072, 16 heads, d=64).

Sharding: 8 cores = 2 batches x 4 head-groups (4 heads each). Each core
computes q/k/v projections + LN for its head group from the full x[b] and
the valid-key-compacted y[b], transposed attention (keys on partitions,
padding mask fused as ACT exp bias), pv with a ones column for softmax
denominators, and its partial output projection. Host sums 4 bf16 partials
per batch and adds bo.

v2 layout of work, tuned so the ACT engine (the exp bottleneck, ~12.7M
exp elems/core at ~1 elem/cycle/lane) streams wall-to-wall:
  - ACT does ONLY exp. LN rsqrt runs on DVE (bitcast magic + 2 Newton
    steps); all PSUM evictions go to vector/gpsimd/scalar-copy (copy is in
    every ACT table set). One exp table load, hoisted to t=0 via a dummy.
  - k-proj + k-LN emitted first (attention gates on full kT), then q;
    v-proj is interleaved into attention block 0's score slots.
  - Softmax normalize without DMA roundtrips: the pv PSUM ones-row is
    broadcast across 64 partitions by a K=1 PE matmul, reciprocal via
    reciprocal_approx_fast, one tensor_mul into ctxT.
  - Output partials in bf16 (halves the 8MB write), DMA'd per i-tile.
"""

import math
import os

import ml_dtypes
import numpy as np

import concourse.bacc as bacc
import concourse.bass as bass
import concourse.tile as tile
from concourse import mybir
from concourse.bass_utils import run_bass_kernel_spmd
from concourse.masks import make_identity

F32 = mybir.dt.float32
I32 = mybir.dt.int32
BF16 = mybir.dt.bfloat16

P = 128
D = 64
EPS = 1e-6
MASK_NEG = -1e9
MAGIC = 0x5F3759DF

LAST_EXEC_NS = None


def _bcast_row(ap, nparts):
    return bass.AP(
        tensor=ap.tensor, offset=ap.offset, ap=[[0, nparts]] + list(ap.ap[1:])
    )


def _build_nc(S1, S2P, C, flags):
    G = 4 * D  # 256 channels per core (4 heads)
    GA = G + 4  # + per-head weight-sum columns (free LN sums via PE)
    NI = S1 // P
    NJ = S2P // P
    CT = C // P
    IBW = 1024
    NIB = S1 // IBW
    NC2 = IBW // 512
    AF = mybir.ActivationFunctionType
    OP = mybir.AluOpType

    nc = bacc.Bacc("TRN2", target_bir_lowering=False, debug=False)

    xT_d = nc.dram_tensor("xT", [C, S1], BF16, kind="ExternalInput")
    yT_d = nc.dram_tensor("yT", [C, S2P], BF16, kind="ExternalInput")
    wqT_d = nc.dram_tensor("wqT", [C, GA], BF16, kind="ExternalInput")
    wkT_d = nc.dram_tensor("wkT", [C, GA], BF16, kind="ExternalInput")
    wvT_d = nc.dram_tensor("wvT", [C, G], BF16, kind="ExternalInput")
    woT_d = nc.dram_tensor("woT", [G, C], BF16, kind="ExternalInput")
    vec_d = nc.dram_tensor("vec", [8, G], F32, kind="ExternalInput")
    mask_d = nc.dram_tensor("maskb", [S2P], F32, kind="ExternalInput")
    out_d = nc.dram_tensor("out", [S1, C], BF16, kind="ExternalOutput")

    VROW = {"bq": 0, "bk": 1, "bv": 2, "qw": 3, "qb": 4, "kw": 5, "kb": 6}

    with tile.TileContext(nc) as tc:
        with (
            tc.tile_pool(name="singles", bufs=1) as singles,
            tc.tile_pool(name="persist", bufs=1) as persist,
        ):
            ident = singles.tile([P, P], BF16, tag="ident")
            mask_sb = singles.tile([P, NJ], F32, tag="mask")
            dummy = singles.tile([P, 1], F32, tag="dummy")
            ones65 = singles.tile([P, D], F32, tag="ones65")
            vec_sb = {}
            for nm in [k for k, use in flags.items() if use]:
                vec_sb[nm] = singles.tile([P, G], F32, tag=f"vec_{nm}", name=f"v{nm}")

            # hoist the exp ACT table load to t=0; ACT then only ever runs
            # exp + Square (Square is in every table set -> no reloads)
            nc.vector.memset(dummy, 0.0)
            nc.scalar.activation(out=dummy, in_=dummy, func=AF.Exp)
            nc.vector.memset(ones65, 1.0)

            qT = [
                persist.tile([P, S1], BF16, tag=f"qT{i}", name=f"qT{i}")
                for i in range(2)
            ]
            kT = [
                persist.tile([P, S2P], BF16, tag=f"kT{i}", name=f"kT{i}")
                for i in range(2)
            ]
            v_sb = persist.tile([P, NJ, 4 * (D + 1)], BF16, tag="v")
            ctxT = [
                persist.tile([P, S1], BF16, tag=f"ctxT{i}", name=f"ctxT{i}")
                for i in range(2)
            ]
            wo_sb = persist.tile([P, 2, C], BF16, tag="wo")
            v4 = v_sb.rearrange("p j (h e) -> p j h e", e=D + 1)
            nc.vector.memset(v4[:, :, :, D : D + 1], 1.0)

            wov = woT_d[:, :].rearrange("(k p) c -> k p c", p=P)

            def newton_rsqrt(eng, work, var, n, tag):
                """rs = 1/sqrt(var) elementwise on [P, n] f32 via bitcast
                magic seed + 2 Newton iterations (~5e-6 rel err)."""
                yi = work.tile([P, n], I32, tag=f"{tag}yi", name="yi")
                vi = var.bitcast(I32)
                eng.tensor_scalar(
                    out=yi, in0=vi, scalar1=1, scalar2=-1,
                    op0=OP.logical_shift_right, op1=OP.bitwise_xor,
                )
                eng.tensor_scalar_add(out=yi, in0=yi, scalar1=MAGIC + 1)
                y = yi.bitcast(F32)
                rs = work.tile([P, n], F32, tag=f"{tag}rs", name="rs")
                t = work.tile([P, n], F32, tag=f"{tag}t", name="t")
                cur = y
                for _ in range(2):
                    eng.tensor_mul(out=t, in0=cur, in1=cur)
                    eng.tensor_mul(out=t, in0=t, in1=var)
                    eng.tensor_scalar(
                        out=t, in0=t, scalar1=-0.5, scalar2=1.5,
                        op0=OP.mult, op1=OP.add,
                    )
                    eng.tensor_mul(out=rs, in0=cur, in1=t)
                    cur = rs
                return rs

            def ln_batch(eng, work, ssum, ssq, ntiles, scale_fold, tag):
                """ssum/ssq [P, ntiles*4] f32 -> rs = scale/sqrt(var+eps),
                nm = -mu*rs (both [P, ntiles*4])."""
                n4 = ntiles * 4
                mu = work.tile([P, n4], F32, tag=f"{tag}mu", name="mu")
                eng.tensor_scalar_mul(out=mu, in0=ssum, scalar1=1.0 / D)
                var = work.tile([P, n4], F32, tag=f"{tag}va", name="var")
                eng.tensor_scalar(
                    out=var, in0=ssq, scalar1=1.0 / D, scalar2=EPS,
                    op0=OP.mult, op1=OP.add,
                )
                m2 = work.tile([P, n4], F32, tag=f"{tag}m2", name="m2")
                eng.tensor_mul(out=m2, in0=mu, in1=mu)
                eng.scalar_tensor_tensor(
                    out=var, in0=m2, scalar=-1.0, in1=var, op0=OP.mult, op1=OP.add
                )
                rs = newton_rsqrt(eng, work, var, n4, tag)
                if scale_fold != 1.0:
                    eng.tensor_scalar_mul(out=rs, in0=rs, scalar1=scale_fold)
                nm = work.tile([P, n4], F32, tag=f"{tag}nm", name="nm")
                eng.scalar_tensor_tensor(
                    out=nm, in0=mu, scalar=-1.0, in1=rs, op0=OP.mult, op1=OP.mult
                )
                return rs, nm

            # ---------------- projections -------------------------------
            phv_cm = tc.tile_pool(name="phv", bufs=1)
            phv = phv_cm.__enter__()
            yT_sb = phv.tile([P, CT, S2P], BF16, tag="yTs")
            wv_sb = phv.tile([P, CT, G], BF16, tag="wvs")
            with (
                tc.tile_pool(name="ph1", bufs=1) as ph1,
                tc.tile_pool(name="work", bufs=3) as work,
                tc.tile_pool(name="psA", bufs=3, space="PSUM") as psA,
                tc.tile_pool(name="psT", bufs=2, space="PSUM") as psT,
            ):
                wq_sb = ph1.tile([P, CT, GA], BF16, tag="wqs")
                wk_sb = ph1.tile([P, CT, GA], BF16, tag="wks")
                wqv = wqT_d[:, :].rearrange("(ct p) g -> ct p g", p=P)
                wkv = wkT_d[:, :].rearrange("(ct p) g -> ct p g", p=P)
                wvv = wvT_d[:, :].rearrange("(ct p) g -> ct p g", p=P)
                nc.gpsimd.dma_start(out=wk_sb, in_=wkv)
                nc.gpsimd.dma_start(out=wv_sb, in_=wvv)
                nc.gpsimd.dma_start(
                    out=mask_sb, in_=mask_d[:].rearrange("(j p) -> p j", p=P)
                )
                nc.gpsimd.dma_start(out=wq_sb, in_=wqv)
                for nm_, t in vec_sb.items():
                    nc.gpsimd.dma_start(
                        out=t, in_=_bcast_row(vec_d[VROW[nm_] : VROW[nm_] + 1, :], P)
                    )
                for kt in range(2):
                    nc.gpsimd.dma_start(out=wo_sb[:, kt, :], in_=wov[kt])
                make_identity(nc, ident)

                yv = yT_d[:, :].rearrange("(ct p) j -> ct p j", p=P)
                xT_sb = ph1.tile([P, CT, S1], BF16, tag="xTs")
                xv = xT_d[:, :].rearrange("(ct p) i -> ct p i", p=P)
                # y first (k gates attention), 512-col blocks covering all ct
                for j0 in range(0, S2P, 512):
                    js = slice(j0, min(j0 + 512, S2P))
                    nc.sync.dma_start(out=yT_sb[:, :, js], in_=yv[:, :, js])
                for i0 in range(0, S1, 512):
                    isl = slice(i0, i0 + 512)
                    nc.scalar.dma_start(out=xT_sb[:, :, isl], in_=xv[:, :, isl])

                qraw = ph1.tile([P, NI, G], BF16, tag="qraw")
                kraw = ph1.tile([P, NJ, G], BF16, tag="kraw")
                qsum = ph1.tile([P, NI * 4], F32, tag="qsum")
                ksum = ph1.tile([P, NJ * 4], F32, tag="ksum")
                qsq = ph1.tile([P, NI, 4, 1], F32, tag="qsq")
                ksq = ph1.tile([P, NJ, 4, 1], F32, tag="ksq")

                def proj_tile(idx, act_sb, w_sb, raw, ssum, ssq, bias_nm, act_ok):
                    ps = psA.tile([P, GA], F32, tag="psA", name="ps")
                    for ct in range(CT):
                        nc.tensor.matmul(
                            ps,
                            lhsT=act_sb[:, ct, idx * P : (idx + 1) * P],
                            rhs=w_sb[:, ct, :],
                            start=(ct == 0),
                            stop=(ct == CT - 1),
                        )
                    dst = raw[:, idx, :]
                    assert bias_nm not in vec_sb, "bias + aug-sum LN unsupported"
                    if idx % 2 == 0:
                        nc.vector.tensor_copy(out=dst, in_=ps[:, 0:G])
                        nc.vector.tensor_copy(
                            out=ssum[:, idx * 4 : idx * 4 + 4], in_=ps[:, G:GA]
                        )
                    else:
                        nc.scalar.copy(out=dst, in_=ps[:, 0:G])
                        nc.scalar.copy(
                            out=ssum[:, idx * 4 : idx * 4 + 4], in_=ps[:, G:GA]
                        )
                    if act_ok and idx % 2 == 0:
                        # sumsq via ACT Square+accum straight from PSUM
                        sqd = work.tile([P, G], F32, tag="sqd", name="sqd")
                        for h in range(4):
                            nc.scalar.activation(
                                out=sqd[:, h * D : (h + 1) * D],
                                in_=ps[:, h * D : (h + 1) * D],
                                func=AF.Square,
                                accum_out=ssq[:, idx, h, :],
                            )
                    else:
                        # sumsq via gpsimd square (SBUF) + vector free-axis reduce
                        sq = work.tile([P, G], F32, tag="sqg", name="sq")
                        nc.gpsimd.tensor_mul(out=sq, in0=dst, in1=dst)
                        nc.vector.tensor_reduce(
                            out=ssq[:, idx, :, :],
                            in_=sq.rearrange("p (h e) -> p h e", e=D),
                            axis=mybir.AxisListType.X,
                            op=OP.add,
                        )

                def apply_transpose(idx, raw, rs, nm, w_nm, b_nm, dstT):
                    qa = work.tile([P, G], BF16, tag="qa", name="qa")
                    is_k = dstT is kT
                    for h4 in range(4):
                        i4 = idx * 4 + h4
                        r = (idx + h4) % 2
                        if is_k and r == 0:
                            # ACT Identity = raw*rs + nm; Identity is in every
                            # table set so this never reloads ACT tables
                            nc.scalar.activation(
                                out=qa[:, h4 * D : (h4 + 1) * D],
                                in_=raw[:, idx, h4 * D : (h4 + 1) * D],
                                func=AF.Identity,
                                scale=rs[:, i4 : i4 + 1],
                                bias=nm[:, i4 : i4 + 1],
                            )
                        else:
                            eng = (nc.gpsimd, nc.vector)[r] if not is_k else nc.gpsimd
                            eng.tensor_scalar(
                                out=qa[:, h4 * D : (h4 + 1) * D],
                                in0=raw[:, idx, h4 * D : (h4 + 1) * D],
                                scalar1=rs[:, i4 : i4 + 1],
                                scalar2=nm[:, i4 : i4 + 1],
                                op0=OP.mult,
                                op1=OP.add,
                            )
                    if w_nm in vec_sb:
                        nc.vector.tensor_mul(out=qa, in0=qa, in1=vec_sb[w_nm])
                    if b_nm in vec_sb:
                        nc.vector.tensor_add(out=qa, in0=qa, in1=vec_sb[b_nm])
                    for half in range(2):
                        pt = psT.tile([P, P], BF16, tag="ptr", name="pt")
                        nc.tensor.transpose(pt, qa[:, half * P : (half + 1) * P], ident)
                        if (idx + half) % 2 == 0:
                            nc.scalar.copy(
                                out=dstT[half][:, idx * P : (idx + 1) * P], in_=pt
                            )
                        else:
                            nc.vector.tensor_copy(
                                out=dstT[half][:, idx * P : (idx + 1) * P], in_=pt
                            )

                # k first: attention can begin once kT + first q block ready
                for jt in range(NJ):
                    proj_tile(jt, yT_sb, wk_sb, kraw, ksum, ksq, "bk", True)
                krs, knm = ln_batch(
                    nc.vector, work, ksum, ksq.rearrange("p t h o -> p (t h o)"),
                    NJ, 1.0, "k",
                )
                for jt in range(NJ):
                    apply_transpose(jt, kraw, krs, knm, "kw", "kb", kT)

                for it in range(NI):
                    proj_tile(it, xT_sb, wq_sb, qraw, qsum, qsq, "bq", it < 8)
                qrs, qnm = ln_batch(
                    nc.vector, work, qsum, qsq.rearrange("p t h o -> p (t h o)"),
                    NI, 1.0 / math.sqrt(D), "q",
                )
                for it in range(NI):
                    apply_transpose(it, qraw, qrs, qnm, "qw", "qb", qT)

            # ---------------- attention + output projection -------------
            with (
                tc.tile_pool(name="pp", bufs=2) as ppool,
                tc.tile_pool(name="attw", bufs=6) as attw,
                tc.tile_pool(name="ow", bufs=3) as ow,
                tc.tile_pool(name="psS", bufs=3, space="PSUM") as psS,
                tc.tile_pool(name="acc", bufs=2, space="PSUM") as accp,
            ):
                blocks = [(ib, hp) for ib in range(NIB) for hp in range(2)]
                pts_store = {}

                def v_proj_jt(jt):
                    ps = accp.tile([P, 512], F32, tag="acc", name="ps")
                    for ct in range(CT):
                        nc.tensor.matmul(
                            ps[:, 0:G],
                            lhsT=yT_sb[:, ct, jt * P : (jt + 1) * P],
                            rhs=wv_sb[:, ct, :],
                            start=(ct == 0),
                            stop=(ct == CT - 1),
                        )
                    ps3 = ps[:, 0:G].rearrange("p (h e) -> p h e", e=D)
                    vdst = v4[:, jt, :, 0:D]
                    if "bv" in vec_sb:
                        bv3 = vec_sb["bv"].rearrange("p (h e) -> p h e", e=D)
                        nc.vector.tensor_add(out=vdst, in0=ps3, in1=bv3)
                    else:
                        nc.vector.tensor_copy(out=vdst, in_=ps3)

                def emit_scores_jt(n, jt, pts, h2s=(0, 1)):
                    ib, hp = blocks[n]
                    for h2 in h2s:
                        ps = psS.tile([P, IBW], F32, tag="ps_s", name="ps")
                        for cc in range(NC2):
                            c0 = ib * IBW + cc * 512
                            nc.tensor.matmul(
                                ps[:, cc * 512 : (cc + 1) * 512],
                                lhsT=kT[hp][
                                    h2 * D : (h2 + 1) * D, jt * P : (jt + 1) * P
                                ],
                                rhs=qT[hp][h2 * D : (h2 + 1) * D, c0 : c0 + 512],
                                start=True,
                                stop=True,
                            )
                        nc.scalar.activation(
                            out=pts[h2][:, jt, :],
                            in_=ps,
                            func=AF.Exp,
                            bias=mask_sb[:, jt : jt + 1],
                            scale=1.0,
                        )

                def normalize_uu(n, state, h2, cc):
                    """pc accumulation for (h2, cc) is complete: evict ctx+den,
                    broadcast den via K=1 matmul, recip, scale into ctxT."""
                    ib, hp = blocks[n]
                    pc = state["pc"]
                    cu = attw.tile([D + 1, 512], F32, tag="cu", name="cu")
                    nc.vector.tensor_copy(out=cu, in_=pc)
                    rb = accp.tile([P, 512], F32, tag="acc", name="rb")
                    nc.tensor.matmul(
                        rb[0:D, :],
                        lhsT=ones65[D : D + 1, 0:D],
                        rhs=cu[D : D + 1, :],
                        start=True,
                        stop=True,
                    )
                    rec = attw.tile([D, 512], F32, tag="rec", name="rec")
                    nc.vector.reciprocal_approx_fast(out=rec, in_=rb[0:D, :])
                    c0 = ib * IBW + cc * 512
                    nc.gpsimd.tensor_mul(
                        out=ctxT[hp][h2 * D : (h2 + 1) * D, c0 : c0 + 512],
                        in0=cu[0:D, :],
                        in1=rec,
                    )

                def emit_ctx_steps(n, state, nsteps):
                    ib, hp = blocks[n]
                    pts = pts_store[n]
                    for _ in range(nsteps):
                        h2, cc, jt = state["pos"]
                        if h2 == 2:
                            return
                        hg = hp * 2 + h2
                        if jt == 0:
                            state["pc"] = accp.tile(
                                [D + 1, 512], F32, tag="acc", name="pc"
                            )
                        nc.tensor.matmul(
                            state["pc"],
                            lhsT=v_sb[:, jt, hg * (D + 1) : (hg + 1) * (D + 1)],
                            rhs=pts[h2][:, jt, cc * 512 : (cc + 1) * 512],
                            start=(jt == 0),
                            stop=(jt == NJ - 1),
                        )
                        if jt == NJ - 1:
                            normalize_uu(n, state, h2, cc)
                            state["pos"] = (h2 + (cc + 1) // NC2, (cc + 1) % NC2, 0)
                        else:
                            state["pos"] = (h2, cc, jt + 1)

                def finish_ctx(n, state):
                    ib, hp = blocks[n]
                    while state["pos"][0] != 2:
                        emit_ctx_steps(n, state, 1000)
                    pts_store.pop(n)
                    if hp == 1:
                        for it in range(ib * IBW // P, (ib + 1) * IBW // P):
                            ot = ow.tile([P, C], BF16, tag="ot", name="ot")
                            for oc in range(C // 512):
                                po = accp.tile([P, 512], F32, tag="acc", name="po")
                                for kt in range(2):
                                    nc.tensor.matmul(
                                        po,
                                        lhsT=ctxT[kt][:, it * P : (it + 1) * P],
                                        rhs=wo_sb[:, kt, oc * 512 : (oc + 1) * 512],
                                        start=(kt == 0),
                                        stop=(kt == 1),
                                    )
                                nc.vector.tensor_copy(
                                    out=ot[:, oc * 512 : (oc + 1) * 512], in_=po
                                )
                            deng = (nc.sync, nc.gpsimd)[it % 2]
                            deng.dma_start(
                                out=out_d[it * P : (it + 1) * P, :], in_=ot
                            )

                def finish_ctx_last(n, state):
                    finish_ctx(n, state)

                def new_pts(n):
                    pts_store[n] = [
                        ppool.tile(
                            [P, NJ, IBW], BF16, tag=f"p{h2}", name=f"p{h2}"
                        )
                        for h2 in range(2)
                    ]
                    return pts_store[n]

                nmm_per_step = (3 * 2 * NC2 * NJ + NJ - 1) // (2 * NJ)
                pts_cur = new_pts(0)
                for jt in range(NJ):
                    emit_scores_jt(0, jt, pts_cur)
                    v_proj_jt(jt)
                for n in range(1, len(blocks) + 1):
                    state = {"pos": (0, 0, 0), "pc": None}
                    if n < len(blocks):
                        pts_next = new_pts(n)
                        last = n == len(blocks) - 1
                        if last:
                            # h2-major for the final block so its own pv can
                            # chase its scores, halving the drain tail
                            for jt in range(NJ):
                                emit_scores_jt(n, jt, pts_next, h2s=(0,))
                                emit_ctx_steps(n - 1, state, nmm_per_step)
                            st_n = {"pos": (0, 0, 0), "pc": None}
                            pts_store[n + 100] = pts_store[n]
                            for jt in range(NJ):
                                emit_scores_jt(n, jt, pts_next, h2s=(1,))
                                emit_ctx_steps(n - 1, state, nmm_per_step)
                                emit_ctx_steps(n, st_n, 2)
                            finish_ctx(n - 1, state)
                            finish_ctx_last(n, st_n)
                            break
                        for jt in range(NJ):
                            emit_scores_jt(n, jt, pts_next)
                            emit_ctx_steps(n - 1, state, nmm_per_step)
                    finish_ctx(n - 1, state)
            phv_cm.__exit__(None, None, None)

    nc.finalize()
    return nc


def _ensure_axon_hooks():
    try:
        import antenv.axon_hooks  # noqa: F401
    except ImportError:
        import sys
        import types

        import antenv  # noqa: F401

        mod = types.ModuleType("antenv.axon_hooks")
        mod._hook = None
        mod.set_axon_ntff_profile_hook = lambda h: setattr(mod, "_hook", h)
        mod.get_axon_ntff_profile_hook = lambda: mod._hook
        sys.modules["antenv.axon_hooks"] = mod


def kernel(x, y, padding_mask, Wq, bq, Wkv, bkv, qn_w, qn_b, kn_w, kn_b, Wo, bo):
    global LAST_EXEC_NS
    _ensure_axon_hooks()
    x = np.asarray(x, dtype=np.float32)
    y = np.asarray(y, dtype=np.float32)
    padding_mask = np.asarray(padding_mask)
    Wq = np.asarray(Wq, dtype=np.float32)
    bq = np.asarray(bq, dtype=np.float32)
    Wkv = np.asarray(Wkv, dtype=np.float32)
    bkv = np.asarray(bkv, dtype=np.float32)
    qn_w = np.asarray(qn_w, dtype=np.float32)
    qn_b = np.asarray(qn_b, dtype=np.float32)
    kn_w = np.asarray(kn_w, dtype=np.float32)
    kn_b = np.asarray(kn_b, dtype=np.float32)
    Wo = np.asarray(Wo, dtype=np.float32)
    bo = np.asarray(bo, dtype=np.float32)

    b, S1, C = x.shape
    assert b == 2 and C % 16 == 0
    d = C // 16
    scale = d**-0.5
    G = 4 * d

    idxs = [np.flatnonzero(padding_mask[bi]) for bi in range(b)]
    s2v = [len(ix) for ix in idxs]
    S2P = max(P, ((max(s2v) + P - 1) // P) * P)

    flags = {
        "bq": bool(np.any(bq)),
        "bk": bool(np.any(bkv[:C])),
        "bv": bool(np.any(bkv[C:])),
        "qw": not bool(np.all(qn_w == 1.0)),
        "qb": bool(np.any(qn_b)),
        "kw": not bool(np.all(kn_w == 1.0)),
        "kb": bool(np.any(kn_b)),
    }

    nc = _build_nc(S1, S2P, C, flags)

    bf = ml_dtypes.bfloat16
    in_maps = []
    yTs = []
    for bi in range(b):
        yv = np.zeros((S2P, C), np.float32)
        yv[: s2v[bi]] = y[bi][idxs[bi]]
        yTs.append(np.ascontiguousarray(yv.T).astype(bf))
    xTs = [np.ascontiguousarray(x[bi].T).astype(bf) for bi in range(b)]
    def aug_sum(wT):
        # append 4 per-head column sums: free per-query LN sums via the PE
        s = wT.reshape(C, 4, d).sum(-1)
        return np.ascontiguousarray(np.concatenate([wT, s], axis=1))

    for core in range(8):
        bc, g = divmod(core, 4)
        rows = slice(g * G, (g + 1) * G)
        vecs = np.zeros((8, G), np.float32)
        vecs[0] = bq[rows]
        vecs[1] = bkv[rows]
        vecs[2] = bkv[C + g * G : C + (g + 1) * G]
        vecs[3] = np.tile(qn_w, 4)
        vecs[4] = np.tile(qn_b * scale, 4)
        vecs[5] = np.tile(kn_w, 4)
        vecs[6] = np.tile(kn_b, 4)
        mb = np.zeros((S2P,), np.float32)
        mb[s2v[bc] :] = MASK_NEG
        in_maps.append(
            {
                "xT": xTs[bc],
                "yT": yTs[bc],
                "wqT": aug_sum(Wq[rows, :].T.astype(np.float32)).astype(bf),
                "wkT": aug_sum(Wkv[:C][rows, :].T.astype(np.float32)).astype(bf),
                "wvT": np.ascontiguousarray(
                    Wkv[C + g * G : C + (g + 1) * G, :].T
                ).astype(bf),
                "woT": np.ascontiguousarray(Wo[:, rows].T).astype(bf),
                "vec": vecs,
                "maskb": mb,
            }
        )

    res = run_bass_kernel_spmd(nc, in_maps, core_ids=list(range(8)))
    LAST_EXEC_NS = res.exec_time_ns

    out = np.zeros((b, S1, C), np.float32)
    for core in range(8):
        out[core // 4] += res.results[core]["out"].astype(np.float32)
    out += bo
    return out
